# revision 60
# baseline (speedup 1.0000x reference)
"""Trainium2 Bass kernel for ContinuousBinaryTreeConvLayer.

Math (per batch b, node n, child slot j in [0,8)):
  m_j   = (children[n,j] != 0)
  s     = sum_j m_j
  H_r[n] = sum_j cr_j * Z[c_j],  S_m[n] = sum_j m_j * Z[c_j]
  out_n = relu(Z[n] @ w_t + H_r @ (w_r - w_l) + S_m @ w_l + bias)

with cr_j = j*m_j/(s-1) for s>=2, cr = 0.5*m_0 at j=0 for s==1, else 0.

Implementation: the child gather/aggregation is NOT a dma_gather (the SWDGE
descriptor generation on the Q7 cores is the bottleneck at ~8 ns/index =
517 us/core for 65536 indices).  Instead the aggregation is expressed as a
dense matmul against host-built window coefficient matrices:

  aggT[f, n] = sum_t  Z_t^T @ C_t[., n]      (t = 16 windows of 128 source
                                              rows, PSUM-accumulated)

where C_t[i, n] = sum of coefficients of slots (n, j) with children[n,j] ==
128*t + i.  C is pure graph-structure preprocessing of `children` (like the
baseline's gather-index relayout), shipped over *affine* DMA at full HBM
bandwidth.  The cr coefficients are factored as cr = scale_n * (j*m_j) so
every C entry ({0, 0.5, 1..7}) is exactly representable in fp16/fp8; the
per-node scale 1/(s-1) is applied on DVE in stage 2 (exact algebra).

Per core (data-parallel over batch: 4 batches/core x 8 cores), each batch
is processed as two 1024-node halves:
  - DMA Z (row-major, fp16) and Z^T (host-transposed, fp16) per batch.
  - Stage 1 (PE): per half: 16 windows x 4 bank-matmuls [K=128, M=128f,
    N=512] accumulating [cr | m] together in one 4-bank PSUM generation
    (C tiles stream in fp8e4 on the sync/scalar HWDGE queues, 16-deep
    prefetch).  Evacuation runs on DVE (cr banks, with the 1/(s-1) scale
    multiply) and ACT (m banks, plain copy) IN PARALLEL.
  - Stage 2 (PE): the half's 8 node-chunks immediately follow, filling the
    PE wait for PSUM-bank reuse by the next half: Z^T@w_t +
    aggT_cr@(w_r-w_l) + aggT_m@w_l + bias in one PSUM group; ACT relu ->
    fp16 out store.

vs the dma_gather baseline (581 us): ~152-161 us on HW (3.8x).  The PE is
saturated (130 us busy, <2 us of gaps: 65536 routed columns/batch at
1 col/cycle warm); the rest is fixed NEFF preamble/tail.  DMA ~120 us busy.
"""

import numpy as np

B, N, C, F, O = 32, 2048, 8, 128, 128
NCORES = 8
BPC = B // NCORES            # batches per core
NWIN = N // 128              # 16 source windows per batch
NBANK = 4                    # 512-col matmuls per 2048-col half

_COMPILED = {}

C_DTYPE = "float8e4"         # coefficient matrix dtype: float16 or float8e4


def _build_nc():
    from contextlib import ExitStack

    import concourse.bacc as bacc
    import concourse.mybir as mybir

    import concourse.tile as tile

    dt = mybir.dt
    Alu = mybir.AluOpType
    cdt = getattr(dt, C_DTYPE)

    nc = bacc.Bacc("TRN2", target_bir_lowering=False, debug=False,
                   num_devices=NCORES)

    z_d = nc.dram_tensor("z", [BPC, N, F], dt.float16, kind="ExternalInput")
    zt_d = nc.dram_tensor("zt", [BPC, F, N], dt.float16, kind="ExternalInput")
    # combined coefficient tiles: per (batch, node-half hb, window t) a
    # [128, 2048] tile whose cols 0:1024 are the cr coefficients and
    # 1024:2048 the m coefficients for nodes [1024*hb, 1024*(hb+1)).
    cc_d = nc.dram_tensor("cc", [BPC, 2, NWIN, 128, 2 * (N // 2)], cdt,
                          kind="ExternalInput")
    scl_d = nc.dram_tensor("scl", [BPC, 128, N], dt.float16,
                           kind="ExternalInput")
    wt_d = nc.dram_tensor("w_t", [F, O], dt.float16, kind="ExternalInput")
    wrl_d = nc.dram_tensor("w_rl", [F, O], dt.float16, kind="ExternalInput")
    wl_d = nc.dram_tensor("w_l", [F, O], dt.float16, kind="ExternalInput")
    b_d = nc.dram_tensor("bias", [O, 1], dt.float16, kind="ExternalInput")
    # output is stored TRANSPOSED [o, n]; the host transposes it back
    out_d = nc.dram_tensor("out", [BPC, O, N], dt.float16,
                           kind="ExternalOutput")

    with tile.TileContext(nc) as tc, ExitStack() as ctx:
        const_pool = ctx.enter_context(tc.tile_pool(name="consts", bufs=1))
        wpool = ctx.enter_context(tc.tile_pool(name="weights", bufs=1))
        zpool = ctx.enter_context(tc.tile_pool(name="z", bufs=2))
        ztpool = ctx.enter_context(tc.tile_pool(name="zt", bufs=2))
        cpool = ctx.enter_context(tc.tile_pool(name="cmat", bufs=16))
        sclpool = ctx.enter_context(tc.tile_pool(name="scl", bufs=2))
        aggpool = ctx.enter_context(tc.tile_pool(name="aggsb", bufs=2))
        opool = ctx.enter_context(tc.tile_pool(name="ostage", bufs=2))
        aggps = ctx.enter_context(
            tc.tile_pool(name="aggps", bufs=1, space="PSUM"))
        otps = ctx.enter_context(
            tc.tile_pool(name="otps", bufs=2, space="PSUM"))

        zero_t = const_pool.tile([128, 512], dt.float16)
        nc.gpsimd.memset(zero_t[:], 0.0)

        wt_sb = wpool.tile([F, O], dt.float16)
        wrl_sb = wpool.tile([F, O], dt.float16)
        wl_sb = wpool.tile([F, O], dt.float16)
        bias_c = wpool.tile([O, 1], dt.float16)
        # cold loads (not needed before ~30 us) go on the idle gpsimd queue
        # so the sync/scalar sequencers start the C stream immediately
        nc.gpsimd.dma_start(wt_sb[:], wt_d.ap())
        nc.gpsimd.dma_start(wrl_sb[:], wrl_d.ap())
        nc.gpsimd.dma_start(wl_sb[:], wl_d.ap())
        nc.gpsimd.dma_start(bias_c[:], b_d.ap())

        for b in range(BPC):
            # z_sb[p, (t f)] = nodes[b, 16p + t, f]; "window" t = row set
            # {n : n % 16 == t} with local index i = n // 16 (host C build
            # uses the same (t, i) = (c % 16, c // 16) decomposition).
            z_sb = zpool.tile([128, NWIN * F], dt.float16)
            nc.scalar.dma_start(
                z_sb[:], z_d.ap()[b].rearrange("(p t) f -> p (t f)", p=128))
            zt_sb = ztpool.tile([128, N], dt.float16)
            nc.gpsimd.dma_start(zt_sb[:], zt_d.ap()[b])
            scl_sb = sclpool.tile([128, N], dt.float16)
            nc.gpsimd.dma_start(scl_sb[:], scl_d.ap()[b])

            # ---- stage 1 + stage 2, per 1024-node half ------------------
            # cr (banks 0-1) and m (banks 2-3) accumulate together; the
            # evacuation then runs on DVE (cr, with 1/(s-1) scale) and ACT
            # (m) in parallel, and this half's stage-2 chunks fill the PE
            # wait for PSUM-bank reuse by the next half.
            agg_sb = aggpool.tile([128, 2 * N], dt.float16)
            for hb in range(2):
                hoff = hb * (N // 2)
                ps = [aggps.tile([128, 512], dt.float32, name=f"aggb{k}")
                      for k in range(NBANK)]
                for t in range(NWIN):
                    c_sb = cpool.tile([128, N], cdt)
                    dma_q = nc.sync if t % 2 == 0 else nc.scalar
                    dma_q.dma_start(c_sb[:], cc_d.ap()[b, hb, t])
                    for k in range(NBANK):
                        nc.tensor.matmul(
                            ps[k][:],
                            z_sb[:, t * F:(t + 1) * F],
                            c_sb[:, k * 512:(k + 1) * 512],
                            start=(t == 0), stop=(t == NWIN - 1))
                for k in (0, 1):
                    dst = agg_sb[:, hoff + k * 512:hoff + (k + 1) * 512]
                    nc.vector.tensor_tensor(
                        dst, ps[k][:],
                        scl_sb[:, hoff + k * 512:hoff + (k + 1) * 512],
                        op=Alu.mult)
                for k in (2, 3):
                    dst = agg_sb[:, N + hoff + (k - 2) * 512:
                                 N + hoff + (k - 1) * 512]
                    nc.scalar.copy(dst, ps[k][:])

                # stage 2, weight-stationary: out^T[o, n] accumulates
                # W_t^T@Z^T + W_l^T@aggT_m + W_rl^T@aggT_cr over this
                # half's 1024 node columns (2 PSUM banks); DVE fuses
                # bias-add (per-partition = per-o) + relu on evacuation.
                ot = [otps.tile([128, 512], dt.float32, name=f"ot{k}")
                      for k in range(2)]
                terms = ((wt_sb, zt_sb, 0), (wl_sb, agg_sb, N),
                         (wrl_sb, agg_sb, 0))
                for ti, (w_sb, rhs_sb, roff) in enumerate(terms):
                    for k in range(2):
                        cols = roff + hoff + k * 512
                        nc.tensor.matmul(ot[k][:], w_sb[:],
                                         rhs_sb[:, cols:cols + 512],
                                         start=(ti == 0),
                                         stop=(ti == len(terms) - 1))
                for k in range(2):
                    ostt = opool.tile([128, 512], dt.float16, name="ostt")
                    nc.vector.scalar_tensor_tensor(
                        ostt[:], ot[k][:], bias_c[:], zero_t[:],
                        op0=Alu.add, op1=Alu.max)
                    nc.sync.dma_start(
                        out_d.ap()[b, :, hoff + k * 512:hoff + (k + 1) * 512],
                        ostt[:])

    nc.compile()
    return nc


def _get_compiled():
    if "nc" not in _COMPILED:
        _COMPILED["nc"] = _build_nc()
    return _COMPILED["nc"]


def _np_cdtype():
    if C_DTYPE == "float16":
        return np.float16
    import ml_dtypes
    return ml_dtypes.float8_e4m3


def _prep_core(nodes_core, children_core, wt16, wrl16, wl16, b16):
    """Host-side prep for one core: fp16 node tables + window coefficient
    matrices (pure index/graph preprocessing of `children`)."""
    cdt = _np_cdtype()
    z16 = np.ascontiguousarray(nodes_core.astype(np.float16))
    zt16 = np.ascontiguousarray(z16.transpose(0, 2, 1))

    H = N // 2
    cc = np.empty((BPC, 2, NWIN, 128, N), dtype=cdt)
    scl = np.empty((BPC, 128, N), dtype=np.float16)
    cols = np.repeat(np.arange(N, dtype=np.int64), C)
    jj = np.arange(C, dtype=np.float64)[None, :]
    for b in range(BPC):
        ch = children_core[b]
        m = (ch != 0).astype(np.float64)
        s = m.sum(1)
        single = s == 1.0
        crw = jj * m
        crw[single, :] = 0.0
        crw[single, 0] = 0.5 * m[single, 0]
        src = ch.astype(np.int64).ravel()
        # (t, i) = (c % 16, c // 16) matches the device z_sb window layout
        flat = ((src % NWIN) * 128 + src // NWIN) * N + cols
        ccr = np.bincount(flat, weights=crw.ravel(),
                          minlength=N * N).reshape(NWIN, 128, N)
        cm = np.bincount(flat, weights=m.ravel(),
                         minlength=N * N).reshape(NWIN, 128, N)
        for hb in range(2):
            cc[b, hb, :, :, 0:H] = ccr[:, :, hb * H:(hb + 1) * H]
            cc[b, hb, :, :, H:N] = cm[:, :, hb * H:(hb + 1) * H]
        sc = np.ones(N, np.float32)
        big = s >= 2.0
        sc[big] = 1.0 / (s[big] - 1.0)
        scl[b] = np.broadcast_to(sc.astype(np.float16)[None, :], (128, N))
    return {
        "z": z16, "zt": zt16, "cc": cc, "scl": scl,
        "w_t": wt16, "w_rl": wrl16, "w_l": wl16, "bias": b16,
    }


def make_in_maps(nodes, children, w_t, w_l, w_r, b):
    nodes = np.asarray(nodes, dtype=np.float32)
    children = np.asarray(children, dtype=np.int32)
    wt16 = np.asarray(w_t, dtype=np.float32).astype(np.float16)
    wrl16 = (np.asarray(w_r, dtype=np.float32)
             - np.asarray(w_l, dtype=np.float32)).astype(np.float16)
    wl16 = np.asarray(w_l, dtype=np.float32).astype(np.float16)
    b16 = np.asarray(b, dtype=np.float32).astype(np.float16).reshape(O, 1)
    in_maps = []
    for core in range(NCORES):
        sl = slice(core * BPC, (core + 1) * BPC)
        in_maps.append(_prep_core(nodes[sl], children[sl],
                                  wt16, wrl16, wl16, b16))
    return in_maps


def kernel(nodes, children, w_t, w_l, w_r, b):
    from concourse.bass_utils import run_bass_kernel_spmd

    nc = _get_compiled()
    in_maps = make_in_maps(nodes, children, w_t, w_l, w_r, b)
    res = run_bass_kernel_spmd(nc, in_maps, core_ids=list(range(NCORES)))
    # device stores out transposed [b, o, n]; undo on host
    out = np.concatenate([res.results[c]["out"].transpose(0, 2, 1)
                          for c in range(NCORES)], axis=0)
    return out.astype(np.float32)


# revision 62
# speedup vs baseline: 1.0065x; 1.0065x over previous
"""Trainium2 Bass kernel for ContinuousBinaryTreeConvLayer.

Math (per batch b, node n, child slot j in [0,8)):
  m_j   = (children[n,j] != 0)
  s     = sum_j m_j
  H_r[n] = sum_j cr_j * Z[c_j],  S_m[n] = sum_j m_j * Z[c_j]
  out_n = relu(Z[n] @ w_t + H_r @ (w_r - w_l) + S_m @ w_l + bias)

with cr_j = j*m_j/(s-1) for s>=2, cr = 0.5*m_0 at j=0 for s==1, else 0.

Implementation: the child gather/aggregation is NOT a dma_gather (the SWDGE
descriptor generation on the Q7 cores is the bottleneck at ~8 ns/index =
517 us/core for 65536 indices).  Instead the aggregation is expressed as a
dense matmul against host-built window coefficient matrices:

  aggT[f, n] = sum_t  Z_t^T @ C_t[., n]      (t = 16 windows of 128 source
                                              rows, PSUM-accumulated)

where C_t[i, n] = sum of coefficients of slots (n, j) with children[n,j] ==
128*t + i.  C is pure graph-structure preprocessing of `children` (like the
baseline's gather-index relayout), shipped over *affine* DMA at full HBM
bandwidth.  The cr coefficients are factored as cr = scale_n * (j*m_j) so
every C entry ({0, 0.5, 1..7}) is exactly representable in fp16/fp8; the
per-node scale 1/(s-1) is applied on DVE in stage 2 (exact algebra).

Per core (data-parallel over batch: 4 batches/core x 8 cores), each batch
is processed as two 1024-node halves:
  - DMA Z (row-major, fp16) and Z^T (host-transposed, fp16) per batch.
  - Stage 1 (PE): per half: 16 windows x 4 bank-matmuls [K=128, M=128f,
    N=512] accumulating [cr | m] together in one 4-bank PSUM generation
    (C tiles stream in fp8e4 on the sync/scalar HWDGE queues, 16-deep
    prefetch).  Evacuation runs on DVE (cr banks, with the 1/(s-1) scale
    multiply) and ACT (m banks, plain copy) IN PARALLEL.
  - Stage 2 (PE): the half's 8 node-chunks immediately follow, filling the
    PE wait for PSUM-bank reuse by the next half: Z^T@w_t +
    aggT_cr@(w_r-w_l) + aggT_m@w_l + bias in one PSUM group; ACT relu ->
    fp16 out store.

vs the dma_gather baseline (581 us): ~152-161 us on HW (3.8x).  The PE is
saturated (130 us busy, <2 us of gaps: 65536 routed columns/batch at
1 col/cycle warm); the rest is fixed NEFF preamble/tail.  DMA ~120 us busy.
"""

import numpy as np

B, N, C, F, O = 32, 2048, 8, 128, 128
NCORES = 8
BPC = B // NCORES            # batches per core
NWIN = N // 128              # 16 source windows per batch
NBANK = 4                    # 512-col matmuls per 2048-col half

_COMPILED = {}

C_DTYPE = "float8e4"         # coefficient matrix dtype: float16 or float8e4


def _build_nc():
    from contextlib import ExitStack

    import concourse.bacc as bacc
    import concourse.mybir as mybir

    import concourse.tile as tile

    dt = mybir.dt
    Alu = mybir.AluOpType
    cdt = getattr(dt, C_DTYPE)

    nc = bacc.Bacc("TRN2", target_bir_lowering=False, debug=False,
                   num_devices=NCORES)

    z_d = nc.dram_tensor("z", [BPC, N, F], dt.float16, kind="ExternalInput")
    zt_d = nc.dram_tensor("zt", [BPC, F, N], dt.float16, kind="ExternalInput")
    # combined coefficient tiles: per (batch, node-half hb, window t) a
    # [128, 2048] tile whose cols 0:1024 are the cr coefficients and
    # 1024:2048 the m coefficients for nodes [1024*hb, 1024*(hb+1)).
    cc_d = nc.dram_tensor("cc", [BPC, 2, NWIN, 128, 2 * (N // 2)], cdt,
                          kind="ExternalInput")
    scl_d = nc.dram_tensor("scl", [BPC, 128, N], dt.float16,
                           kind="ExternalInput")
    wt_d = nc.dram_tensor("w_t", [F, O], dt.float16, kind="ExternalInput")
    wrl_d = nc.dram_tensor("w_rl", [F, O], dt.float16, kind="ExternalInput")
    wl_d = nc.dram_tensor("w_l", [F, O], dt.float16, kind="ExternalInput")
    b_d = nc.dram_tensor("bias", [O, 1], dt.float16, kind="ExternalInput")
    # output is stored TRANSPOSED [o, n]; the host transposes it back
    out_d = nc.dram_tensor("out", [BPC, O, N], dt.float16,
                           kind="ExternalOutput")

    with tile.TileContext(nc) as tc, ExitStack() as ctx:
        const_pool = ctx.enter_context(tc.tile_pool(name="consts", bufs=1))
        wpool = ctx.enter_context(tc.tile_pool(name="weights", bufs=1))
        zpool = ctx.enter_context(tc.tile_pool(name="z", bufs=2))
        ztpool = ctx.enter_context(tc.tile_pool(name="zt", bufs=2))
        cpool = ctx.enter_context(tc.tile_pool(name="cmat", bufs=16))
        sclpool = ctx.enter_context(tc.tile_pool(name="scl", bufs=2))
        aggpool = ctx.enter_context(tc.tile_pool(name="aggsb", bufs=2))
        opool = ctx.enter_context(tc.tile_pool(name="ostage", bufs=2))
        aggps = ctx.enter_context(
            tc.tile_pool(name="aggps", bufs=1, space="PSUM"))
        otps = ctx.enter_context(
            tc.tile_pool(name="otps", bufs=2, space="PSUM"))

        zero_t = const_pool.tile([128, 512], dt.float16)
        nc.gpsimd.memset(zero_t[:], 0.0)

        wt_sb = wpool.tile([F, O], dt.float16)
        wrl_sb = wpool.tile([F, O], dt.float16)
        wl_sb = wpool.tile([F, O], dt.float16)
        bias_c = wpool.tile([O, 1], dt.float16)
        nc.sync.dma_start(wt_sb[:], wt_d.ap())
        nc.sync.dma_start(wrl_sb[:], wrl_d.ap())
        nc.sync.dma_start(wl_sb[:], wl_d.ap())
        nc.sync.dma_start(bias_c[:], b_d.ap())

        for b in range(BPC):
            # z_sb[p, (t f)] = nodes[b, 16p + t, f]; "window" t = row set
            # {n : n % 16 == t} with local index i = n // 16 (host C build
            # uses the same (t, i) = (c % 16, c // 16) decomposition).
            z_sb = zpool.tile([128, NWIN * F], dt.float16)
            nc.scalar.dma_start(
                z_sb[:], z_d.ap()[b].rearrange("(p t) f -> p (t f)", p=128))
            zt_sb = ztpool.tile([128, N], dt.float16)
            nc.scalar.dma_start(zt_sb[:], zt_d.ap()[b])
            scl_sb = sclpool.tile([128, N], dt.float16)
            nc.scalar.dma_start(scl_sb[:], scl_d.ap()[b])

            # ---- stage 1 + stage 2, per 1024-node half ------------------
            # cr (banks 0-1) and m (banks 2-3) accumulate together; the
            # evacuation then runs on DVE (cr, with 1/(s-1) scale) and ACT
            # (m) in parallel, and this half's stage-2 chunks fill the PE
            # wait for PSUM-bank reuse by the next half.
            agg_sb = aggpool.tile([128, 2 * N], dt.float16)
            for hb in range(2):
                hoff = hb * (N // 2)
                ps = [aggps.tile([128, 512], dt.float32, name=f"aggb{k}")
                      for k in range(NBANK)]
                for t in range(NWIN):
                    c_sb = cpool.tile([128, N], cdt)
                    dma_q = nc.sync if t % 2 == 0 else nc.scalar
                    dma_q.dma_start(c_sb[:], cc_d.ap()[b, hb, t])
                    for k in range(NBANK):
                        nc.tensor.matmul(
                            ps[k][:],
                            z_sb[:, t * F:(t + 1) * F],
                            c_sb[:, k * 512:(k + 1) * 512],
                            start=(t == 0), stop=(t == NWIN - 1))
                for k in (0, 1):
                    dst = agg_sb[:, hoff + k * 512:hoff + (k + 1) * 512]
                    nc.vector.tensor_tensor(
                        dst, ps[k][:],
                        scl_sb[:, hoff + k * 512:hoff + (k + 1) * 512],
                        op=Alu.mult)
                for k in (2, 3):
                    dst = agg_sb[:, N + hoff + (k - 2) * 512:
                                 N + hoff + (k - 1) * 512]
                    nc.scalar.copy(dst, ps[k][:])

                # stage 2, weight-stationary: out^T[o, n] accumulates
                # W_t^T@Z^T + W_l^T@aggT_m + W_rl^T@aggT_cr over this
                # half's 1024 node columns (2 PSUM banks); DVE fuses
                # bias-add (per-partition = per-o) + relu on evacuation.
                ot = [otps.tile([128, 512], dt.float32, name=f"ot{k}")
                      for k in range(2)]
                terms = ((wt_sb, zt_sb, 0), (wl_sb, agg_sb, N),
                         (wrl_sb, agg_sb, 0))
                for ti, (w_sb, rhs_sb, roff) in enumerate(terms):
                    for k in range(2):
                        cols = roff + hoff + k * 512
                        nc.tensor.matmul(ot[k][:], w_sb[:],
                                         rhs_sb[:, cols:cols + 512],
                                         start=(ti == 0),
                                         stop=(ti == len(terms) - 1))
                for k in range(2):
                    ostt = opool.tile([128, 512], dt.float16, name="ostt")
                    nc.vector.scalar_tensor_tensor(
                        ostt[:], ot[k][:], bias_c[:], zero_t[:],
                        op0=Alu.add, op1=Alu.max)
                    nc.sync.dma_start(
                        out_d.ap()[b, :, hoff + k * 512:hoff + (k + 1) * 512],
                        ostt[:])

    nc.compile()
    return nc


def _get_compiled():
    if "nc" not in _COMPILED:
        _COMPILED["nc"] = _build_nc()
    return _COMPILED["nc"]


def _np_cdtype():
    if C_DTYPE == "float16":
        return np.float16
    import ml_dtypes
    return ml_dtypes.float8_e4m3


def _prep_core(nodes_core, children_core, wt16, wrl16, wl16, b16):
    """Host-side prep for one core: fp16 node tables + window coefficient
    matrices (pure index/graph preprocessing of `children`)."""
    cdt = _np_cdtype()
    z16 = np.ascontiguousarray(nodes_core.astype(np.float16))
    zt16 = np.ascontiguousarray(z16.transpose(0, 2, 1))

    H = N // 2
    cc = np.empty((BPC, 2, NWIN, 128, N), dtype=cdt)
    scl = np.empty((BPC, 128, N), dtype=np.float16)
    cols = np.repeat(np.arange(N, dtype=np.int64), C)
    jj = np.arange(C, dtype=np.float64)[None, :]
    for b in range(BPC):
        ch = children_core[b]
        m = (ch != 0).astype(np.float64)
        s = m.sum(1)
        single = s == 1.0
        crw = jj * m
        crw[single, :] = 0.0
        crw[single, 0] = 0.5 * m[single, 0]
        src = ch.astype(np.int64).ravel()
        # (t, i) = (c % 16, c // 16) matches the device z_sb window layout
        flat = ((src % NWIN) * 128 + src // NWIN) * N + cols
        ccr = np.bincount(flat, weights=crw.ravel(),
                          minlength=N * N).reshape(NWIN, 128, N)
        cm = np.bincount(flat, weights=m.ravel(),
                         minlength=N * N).reshape(NWIN, 128, N)
        for hb in range(2):
            cc[b, hb, :, :, 0:H] = ccr[:, :, hb * H:(hb + 1) * H]
            cc[b, hb, :, :, H:N] = cm[:, :, hb * H:(hb + 1) * H]
        sc = np.ones(N, np.float32)
        big = s >= 2.0
        sc[big] = 1.0 / (s[big] - 1.0)
        scl[b] = np.broadcast_to(sc.astype(np.float16)[None, :], (128, N))
    return {
        "z": z16, "zt": zt16, "cc": cc, "scl": scl,
        "w_t": wt16, "w_rl": wrl16, "w_l": wl16, "bias": b16,
    }


def make_in_maps(nodes, children, w_t, w_l, w_r, b):
    nodes = np.asarray(nodes, dtype=np.float32)
    children = np.asarray(children, dtype=np.int32)
    wt16 = np.asarray(w_t, dtype=np.float32).astype(np.float16)
    wrl16 = (np.asarray(w_r, dtype=np.float32)
             - np.asarray(w_l, dtype=np.float32)).astype(np.float16)
    wl16 = np.asarray(w_l, dtype=np.float32).astype(np.float16)
    b16 = np.asarray(b, dtype=np.float32).astype(np.float16).reshape(O, 1)
    in_maps = []
    for core in range(NCORES):
        sl = slice(core * BPC, (core + 1) * BPC)
        in_maps.append(_prep_core(nodes[sl], children[sl],
                                  wt16, wrl16, wl16, b16))
    return in_maps


def kernel(nodes, children, w_t, w_l, w_r, b):
    from concourse.bass_utils import run_bass_kernel_spmd

    nc = _get_compiled()
    in_maps = make_in_maps(nodes, children, w_t, w_l, w_r, b)
    res = run_bass_kernel_spmd(nc, in_maps, core_ids=list(range(NCORES)))
    # device stores out transposed [b, o, n]; undo on host
    out = np.concatenate([res.results[c]["out"].transpose(0, 2, 1)
                          for c in range(NCORES)], axis=0)
    return out.astype(np.float32)


# revision 67
# speedup vs baseline: 1.0158x; 1.0092x over previous
"""Trainium2 Bass kernel for ContinuousBinaryTreeConvLayer.

Math (per batch b, node n, child slot j in [0,8)):
  m_j   = (children[n,j] != 0)
  s     = sum_j m_j
  H_r[n] = sum_j cr_j * Z[c_j],  S_m[n] = sum_j m_j * Z[c_j]
  out_n = relu(Z[n] @ w_t + H_r @ (w_r - w_l) + S_m @ w_l + bias)

with cr_j = j*m_j/(s-1) for s>=2, cr = 0.5*m_0 at j=0 for s==1, else 0.

Implementation: the child gather/aggregation is NOT a dma_gather (the SWDGE
descriptor generation on the Q7 cores is the bottleneck at ~8 ns/index =
517 us/core for 65536 indices).  Instead the aggregation is expressed as a
dense matmul against host-built window coefficient matrices:

  aggT[f, n] = sum_t  Z_t^T @ C_t[., n]      (t = 16 windows of 128 source
                                              rows, PSUM-accumulated)

where C_t[i, n] = sum of coefficients of slots (n, j) with children[n,j] ==
128*t + i.  C is pure graph-structure preprocessing of `children` (like the
baseline's gather-index relayout), shipped over *affine* DMA at full HBM
bandwidth.  The cr coefficients are factored as cr = scale_n * (j*m_j) so
every C entry ({0, 0.5, 1..7}) is exactly representable in fp16/fp8; the
per-node scale 1/(s-1) is applied on DVE in stage 2 (exact algebra).

Per core (data-parallel over batch: 4 batches/core x 8 cores), each batch
is processed as two 1024-node halves:
  - DMA Z (row-major, fp16) and Z^T (host-transposed, fp16) per batch.
  - Stage 1 (PE): per half: 16 windows x 4 bank-matmuls [K=128, M=128f,
    N=512] accumulating [cr | m] together in one 4-bank PSUM generation
    (C tiles stream in fp8e4 on the sync/scalar HWDGE queues, 16-deep
    prefetch).  Evacuation runs on DVE (cr banks, with the 1/(s-1) scale
    multiply) and ACT (m banks, plain copy) IN PARALLEL.
  - Stage 2 (PE), weight-stationary: out^T[o, n] = W_t^T@Z^T +
    W_l^T@aggT_m + W_rl^T@aggT_cr accumulated over the half's 1024 node
    columns (2 bank-matmuls of N=512 per term; all three rhs operands are
    already feature-major in SBUF).  It immediately follows the half's
    evacuation, filling the PE wait for PSUM-bank reuse by the next half.
    DVE fuses bias-add (per-partition = per-o) + relu in one
    scalar_tensor_tensor (add, max-with-zero); out is stored TRANSPOSED
    [o, n] fp16 and the host undoes the transpose.

vs the dma_gather baseline (581 us): ~148-158 us on HW (3.9x).  The PE is
saturated (125 us busy, <1 us of gaps: 65536 routed + 12288 stage-2
columns per batch at 1 col/cycle warm); the rest is fixed NEFF
preamble (~9 us) and the final relu/store/finalize tail.  DMA ~116 us busy.
"""

import numpy as np

B, N, C, F, O = 32, 2048, 8, 128, 128
NCORES = 8
BPC = B // NCORES            # batches per core
NWIN = N // 128              # 16 source windows per batch
NBANK = 4                    # 512-col matmuls per 2048-col half

_COMPILED = {}

C_DTYPE = "float8e4"         # coefficient matrix dtype: float16 or float8e4


def _build_nc():
    from contextlib import ExitStack

    import concourse.bacc as bacc
    import concourse.mybir as mybir

    import concourse.tile as tile

    dt = mybir.dt
    Alu = mybir.AluOpType
    cdt = getattr(dt, C_DTYPE)

    nc = bacc.Bacc("TRN2", target_bir_lowering=False, debug=False,
                   num_devices=NCORES)

    z_d = nc.dram_tensor("z", [BPC, N, F], dt.float16, kind="ExternalInput")
    zt_d = nc.dram_tensor("zt", [BPC, F, N], dt.float16, kind="ExternalInput")
    # combined coefficient tiles: per (batch, node-half hb, window t) a
    # [128, 2048] tile whose cols 0:1024 are the cr coefficients and
    # 1024:2048 the m coefficients for nodes [1024*hb, 1024*(hb+1)).
    cc_d = nc.dram_tensor("cc", [BPC, 2, NWIN, 128, 2 * (N // 2)], cdt,
                          kind="ExternalInput")
    scl_d = nc.dram_tensor("scl", [BPC, 128, N], dt.float16,
                           kind="ExternalInput")
    wt_d = nc.dram_tensor("w_t", [F, O], dt.float16, kind="ExternalInput")
    wrl_d = nc.dram_tensor("w_rl", [F, O], dt.float16, kind="ExternalInput")
    wl_d = nc.dram_tensor("w_l", [F, O], dt.float16, kind="ExternalInput")
    b_d = nc.dram_tensor("bias", [O, 1], dt.float16, kind="ExternalInput")
    # output is stored TRANSPOSED [o, n]; the host transposes it back
    out_d = nc.dram_tensor("out", [BPC, O, N], dt.float16,
                           kind="ExternalOutput")

    with tile.TileContext(nc) as tc, ExitStack() as ctx:
        const_pool = ctx.enter_context(tc.tile_pool(name="consts", bufs=1))
        wpool = ctx.enter_context(tc.tile_pool(name="weights", bufs=1))
        zpool = ctx.enter_context(tc.tile_pool(name="z", bufs=2))
        ztpool = ctx.enter_context(tc.tile_pool(name="zt", bufs=2))
        cpool = ctx.enter_context(tc.tile_pool(name="cmat", bufs=24))
        sclpool = ctx.enter_context(tc.tile_pool(name="scl", bufs=2))
        aggpool = ctx.enter_context(tc.tile_pool(name="aggsb", bufs=2))
        opool = ctx.enter_context(tc.tile_pool(name="ostage", bufs=2))
        aggps = ctx.enter_context(
            tc.tile_pool(name="aggps", bufs=1, space="PSUM"))
        otps = ctx.enter_context(
            tc.tile_pool(name="otps", bufs=2, space="PSUM"))

        zero_t = const_pool.tile([128, 512], dt.float16)
        nc.gpsimd.memset(zero_t[:], 0.0)

        # weight/bias loads are EMITTED after the first half-batch's C
        # stream (they are not consumed before ~35 us, and putting them
        # first would delay the first routing matmul by ~4 us of sync-queue
        # issue slots)
        wt_sb = wpool.tile([F, O], dt.float16)
        wrl_sb = wpool.tile([F, O], dt.float16)
        wl_sb = wpool.tile([F, O], dt.float16)
        bias_c = wpool.tile([O, 1], dt.float16)

        def load_weights():
            nc.sync.dma_start(wt_sb[:], wt_d.ap())
            nc.sync.dma_start(wrl_sb[:], wrl_d.ap())
            nc.sync.dma_start(wl_sb[:], wl_d.ap())
            nc.sync.dma_start(bias_c[:], b_d.ap())

        for b in range(BPC):
            # z_sb[p, (t f)] = nodes[b, 16p + t, f]; "window" t = row set
            # {n : n % 16 == t} with local index i = n // 16 (host C build
            # uses the same (t, i) = (c % 16, c // 16) decomposition).
            # priority-load window 0 (256 B/partition) so the first routing
            # matmul of the batch doesn't wait for the full z transfer
            z_sb = zpool.tile([128, NWIN * F], dt.float16)
            z_src = z_d.ap()[b].rearrange("(p t) f -> p (t f)", p=128)
            nc.scalar.dma_start(z_sb[:, 0:F], z_src[:, 0:F])
            nc.scalar.dma_start(z_sb[:, F:], z_src[:, F:])
            zt_sb = ztpool.tile([128, N], dt.float16)
            nc.scalar.dma_start(zt_sb[:], zt_d.ap()[b])
            scl_sb = sclpool.tile([128, N], dt.float16)
            nc.scalar.dma_start(scl_sb[:], scl_d.ap()[b])

            # ---- stage 1 + stage 2, per 1024-node half ------------------
            # cr (banks 0-1) and m (banks 2-3) accumulate together; the
            # evacuation then runs on DVE (cr, with 1/(s-1) scale) and ACT
            # (m) in parallel, and this half's stage-2 chunks fill the PE
            # wait for PSUM-bank reuse by the next half.
            agg_sb = aggpool.tile([128, 2 * N], dt.float16)
            for hb in range(2):
                hoff = hb * (N // 2)
                ps = [aggps.tile([128, 512], dt.float32, name=f"aggb{k}")
                      for k in range(NBANK)]
                for t in range(NWIN):
                    c_sb = cpool.tile([128, N], cdt)
                    dma_q = nc.sync if t % 2 == 0 else nc.scalar
                    dma_q.dma_start(c_sb[:], cc_d.ap()[b, hb, t])
                    for k in range(NBANK):
                        nc.tensor.matmul(
                            ps[k][:],
                            z_sb[:, t * F:(t + 1) * F],
                            c_sb[:, k * 512:(k + 1) * 512],
                            start=(t == 0), stop=(t == NWIN - 1))
                if b == 0 and hb == 0:
                    load_weights()
                for k in (0, 1):
                    dst = agg_sb[:, hoff + k * 512:hoff + (k + 1) * 512]
                    nc.vector.tensor_tensor(
                        dst, ps[k][:],
                        scl_sb[:, hoff + k * 512:hoff + (k + 1) * 512],
                        op=Alu.mult)
                for k in (2, 3):
                    dst = agg_sb[:, N + hoff + (k - 2) * 512:
                                 N + hoff + (k - 1) * 512]
                    nc.scalar.copy(dst, ps[k][:])

                # stage 2, weight-stationary: out^T[o, n] accumulates
                # W_t^T@Z^T + W_l^T@aggT_m + W_rl^T@aggT_cr over this
                # half's 1024 node columns (2 PSUM banks); DVE fuses
                # bias-add (per-partition = per-o) + relu on evacuation.
                ot = [otps.tile([128, 512], dt.float32, name=f"ot{k}")
                      for k in range(2)]
                terms = ((wt_sb, zt_sb, 0), (wl_sb, agg_sb, N),
                         (wrl_sb, agg_sb, 0))
                for ti, (w_sb, rhs_sb, roff) in enumerate(terms):
                    for k in range(2):
                        cols = roff + hoff + k * 512
                        nc.tensor.matmul(ot[k][:], w_sb[:],
                                         rhs_sb[:, cols:cols + 512],
                                         start=(ti == 0),
                                         stop=(ti == len(terms) - 1))
                for k in range(2):
                    ostt = opool.tile([128, 512], dt.float16, name="ostt")
                    nc.vector.scalar_tensor_tensor(
                        ostt[:], ot[k][:], bias_c[:], zero_t[:],
                        op0=Alu.add, op1=Alu.max)
                    nc.sync.dma_start(
                        out_d.ap()[b, :, hoff + k * 512:hoff + (k + 1) * 512],
                        ostt[:])

    nc.compile()
    return nc


def _get_compiled():
    if "nc" not in _COMPILED:
        _COMPILED["nc"] = _build_nc()
    return _COMPILED["nc"]


def _np_cdtype():
    if C_DTYPE == "float16":
        return np.float16
    import ml_dtypes
    return ml_dtypes.float8_e4m3


def _prep_core(nodes_core, children_core, wt16, wrl16, wl16, b16):
    """Host-side prep for one core: fp16 node tables + window coefficient
    matrices (pure index/graph preprocessing of `children`)."""
    cdt = _np_cdtype()
    z16 = np.ascontiguousarray(nodes_core.astype(np.float16))
    zt16 = np.ascontiguousarray(z16.transpose(0, 2, 1))

    H = N // 2
    cc = np.empty((BPC, 2, NWIN, 128, N), dtype=cdt)
    scl = np.empty((BPC, 128, N), dtype=np.float16)
    cols = np.repeat(np.arange(N, dtype=np.int64), C)
    jj = np.arange(C, dtype=np.float64)[None, :]
    for b in range(BPC):
        ch = children_core[b]
        m = (ch != 0).astype(np.float64)
        s = m.sum(1)
        single = s == 1.0
        crw = jj * m
        crw[single, :] = 0.0
        crw[single, 0] = 0.5 * m[single, 0]
        src = ch.astype(np.int64).ravel()
        # (t, i) = (c % 16, c // 16) matches the device z_sb window layout
        flat = ((src % NWIN) * 128 + src // NWIN) * N + cols
        ccr = np.bincount(flat, weights=crw.ravel(),
                          minlength=N * N).reshape(NWIN, 128, N)
        cm = np.bincount(flat, weights=m.ravel(),
                         minlength=N * N).reshape(NWIN, 128, N)
        for hb in range(2):
            cc[b, hb, :, :, 0:H] = ccr[:, :, hb * H:(hb + 1) * H]
            cc[b, hb, :, :, H:N] = cm[:, :, hb * H:(hb + 1) * H]
        sc = np.ones(N, np.float32)
        big = s >= 2.0
        sc[big] = 1.0 / (s[big] - 1.0)
        scl[b] = np.broadcast_to(sc.astype(np.float16)[None, :], (128, N))
    return {
        "z": z16, "zt": zt16, "cc": cc, "scl": scl,
        "w_t": wt16, "w_rl": wrl16, "w_l": wl16, "bias": b16,
    }


def make_in_maps(nodes, children, w_t, w_l, w_r, b):
    nodes = np.asarray(nodes, dtype=np.float32)
    children = np.asarray(children, dtype=np.int32)
    wt16 = np.asarray(w_t, dtype=np.float32).astype(np.float16)
    wrl16 = (np.asarray(w_r, dtype=np.float32)
             - np.asarray(w_l, dtype=np.float32)).astype(np.float16)
    wl16 = np.asarray(w_l, dtype=np.float32).astype(np.float16)
    b16 = np.asarray(b, dtype=np.float32).astype(np.float16).reshape(O, 1)
    in_maps = []
    for core in range(NCORES):
        sl = slice(core * BPC, (core + 1) * BPC)
        in_maps.append(_prep_core(nodes[sl], children[sl],
                                  wt16, wrl16, wl16, b16))
    return in_maps


def kernel(nodes, children, w_t, w_l, w_r, b):
    from concourse.bass_utils import run_bass_kernel_spmd

    nc = _get_compiled()
    in_maps = make_in_maps(nodes, children, w_t, w_l, w_r, b)
    res = run_bass_kernel_spmd(nc, in_maps, core_ids=list(range(NCORES)))
    # device stores out transposed [b, o, n]; undo on host
    out = np.concatenate([res.results[c]["out"].transpose(0, 2, 1)
                          for c in range(NCORES)], axis=0)
    return out.astype(np.float32)
